# revision 5
# baseline (speedup 1.0000x reference)
"""Trainium2 Bass kernel for Expansion + CPSDropout.

Computes, for x[4,256,64,64] f32 and rand_vals[320,320] f32:
    xp   = zero-pad x spatially by 2            -> [b,c,68,68]
    out[b,c,5i+p,5j+q] = xp[b,c,i+p,j+q] * M[5i+p,5j+q]
    M    = (rand_vals > 0.25, forced True at [2::5,2::5]) / 0.75

Strategy (8 cores, data parallel over the 1024 (b,c) channels, 128/core),
v2 — output-row-on-partition layout, all-bf16, 2x DVE mode:
  - host folds the 1/0.75 scale into x and converts to bf16 (one rounding,
    ~0.2% rel err, well under the 2e-2 gate), transposes to [h, n, w] so the
    device can load rows-of-all-images contiguously; mask stays a binary
    bf16 0/1 tensor; a tiny 0/1 selection matrix S maps padded input rows
    to output rows.
  - output rows I = 5i+p live on PARTITIONS (chunks of 128).  The dropout
    mask then needs NO cross-partition broadcast: mask rows DMA directly
    into [128, 320] SBUF tiles.
  - PE matmul xd = S^T @ xT duplicates input row (I//5 + I%5) onto each
    partition I (0/1 weights, exact in bf16), h-padding falls out of S's
    zero columns.  ACT copies PSUM f32 -> SBUF bf16 with a 68-stride so
    w-padding borders (memset by Pool) survive.
  - single full-size pass on DVE: tensor_tensor multiply with the
    w-expansion folded into in0's access pattern
        out[I, n, 5j+q] = xd[I, n, j+q] * m[I, 5j+q]
    All three operands bf16/SBUF/packed-innermost -> DVE 2x_1p perf mode
    (0.52 ns/elem vs 1.04).
  - stores stream 26.2 MB/core of bf16 (vs 52.4 MB f32 before); host
    upcasts to f32 at the end.  DMA-bound at ~60-75us/core.
"""

import numpy as np
import ml_dtypes

import concourse.bass as bass
import concourse.bacc as bacc
import concourse.mybir as mybir
import concourse.tile as tile
from concourse.bass_utils import run_bass_kernel_spmd

N_CORES = 8
N = 128            # images (b*c slices) per core
H = W = 64
S = 5              # stride
S2 = S // 2        # pad = 2
OUT_HW = H * S     # 320
RATE = 0.25
SCALE = float(np.float32(1.0) / np.float32(1.0 - RATE))

NB = 32                      # images per block
N_BLOCKS = N // NB           # 4
CHUNKS = (128, 128, 64)      # output-row chunks over 320 rows
XD_F = NB * (W + 2 * S2)     # 32*68 = 2176 bf16 per partition
OB_F = NB * OUT_HW           # 32*320 = 10240

_CACHE = {}


def _build_nc():
    nc = bacc.Bacc("TRN2", target_bir_lowering=False)
    xh_t = nc.dram_tensor("xh", [H, N * W], mybir.dt.bfloat16, kind="ExternalInput")
    s_t = nc.dram_tensor("sel", [H, 3 * 128], mybir.dt.bfloat16, kind="ExternalInput")
    m_t = nc.dram_tensor(
        "mask", [OUT_HW, OUT_HW], mybir.dt.bfloat16, kind="ExternalInput"
    )
    # I-major so each partition's store is one contiguous 20.5KB chunk
    # (J-major [n, I, J] layout would mean 640B descriptors -> ~280 GB/s);
    # host un-permutes to [n, I, J].
    o_t = nc.dram_tensor(
        "out", [OUT_HW, N, OUT_HW], mybir.dt.bfloat16, kind="ExternalOutput"
    )

    with tile.TileContext(nc) as tc:
        with (
            tc.tile_pool(name="const", bufs=1) as constp,
            tc.tile_pool(name="xbuf", bufs=1) as xbufp,
            tc.tile_pool(name="xd", bufs=3) as xdp,
            tc.tile_pool(name="obuf", bufs=3) as obufp,
            tc.tile_pool(name="mm", bufs=2, space="PSUM") as psump,
        ):
            xT = xbufp.tile([H, N * W], mybir.dt.bfloat16)
            nc.gpsimd.dma_start(out=xT[:], in_=xh_t[:])
            s_sb = constp.tile([H, 3 * 128], mybir.dt.bfloat16)
            nc.gpsimd.dma_start(out=s_sb[:], in_=s_t[:])
            m_sb = constp.tile([128, 3 * OUT_HW], mybir.dt.bfloat16)
            for c, wc in enumerate(CHUNKS):
                nc.gpsimd.dma_start(
                    out=m_sb[0:wc, c * OUT_HW : (c + 1) * OUT_HW],
                    in_=m_t[128 * c : 128 * c + wc, :],
                )

            xd_pstride = None
            for c, wc in enumerate(CHUNKS):
                for b in range(N_BLOCKS):
                    ps = psump.tile([128, NB * W], mybir.dt.float32)
                    for k in range(NB * W // 512):
                        nc.tensor.matmul(
                            ps[0:wc, k * 512 : (k + 1) * 512],
                            s_sb[:, c * 128 : c * 128 + wc],
                            xT[:, b * NB * W + k * 512 : b * NB * W + (k + 1) * 512],
                            start=True,
                            stop=True,
                        )
                    xd = xdp.tile([128, XD_F], mybir.dt.bfloat16)
                    xd_ap = xd[:]
                    # w-pad borders: cols {0,1} and {66,67} of each 68-col row
                    for off in (0, W + S2):
                        nc.gpsimd.memset(
                            bass.AP(
                                tensor=xd_ap.tensor,
                                offset=xd_ap.offset + off,
                                ap=[[XD_F, wc], [W + 2 * S2, NB], [1, S2]],
                            ),
                            0.0,
                        )
                    # PSUM f32 -> SBUF bf16, interior cols (stride 68)
                    nc.scalar.copy(
                        out=bass.AP(
                            tensor=xd_ap.tensor,
                            offset=xd_ap.offset + S2,
                            ap=[[XD_F, wc], [W + 2 * S2, NB], [1, W]],
                        ),
                        in_=ps[0:wc, :].rearrange("p (n w) -> p n w", n=NB),
                    )
                    ob = obufp.tile([128, OB_F], mybir.dt.bfloat16)
                    ob_ap = ob[:]
                    m_ap = m_sb[:]
                    # out[I, n, 5j+q] = xd[I, n, j+q] * m[I, 5j+q]
                    nc.vector.tensor_tensor(
                        out=bass.AP(
                            tensor=ob_ap.tensor,
                            offset=ob_ap.offset,
                            ap=[[OB_F, wc], [OUT_HW, NB], [S, W], [1, S]],
                        ),
                        in0=bass.AP(
                            tensor=xd_ap.tensor,
                            offset=xd_ap.offset,
                            ap=[[XD_F, wc], [W + 2 * S2, NB], [1, W], [1, S]],
                        ),
                        in1=bass.AP(
                            tensor=m_ap.tensor,
                            offset=m_ap.offset + c * OUT_HW,
                            ap=[[3 * OUT_HW, wc], [0, NB], [S, W], [1, S]],
                        ),
                        op=mybir.AluOpType.mult,
                    )
                    dst = o_t[128 * c : 128 * c + wc, b * NB : (b + 1) * NB, :]
                    nc.sync.dma_start(
                        out=dst,
                        in_=ob[0:wc, :].rearrange("p (n J) -> p n J", n=NB),
                    )
    nc.compile()
    return nc


def _get_nc():
    if "nc" not in _CACHE:
        _CACHE["nc"] = _build_nc()
    return _CACHE["nc"]


def _build_sel() -> np.ndarray:
    """S[r', c*128+m] = 1 iff padded row of output-row I=c*128+m is r'+2."""
    sel = np.zeros((H, 3 * 128), dtype=np.float32)
    for c, wc in enumerate(CHUNKS):
        for m in range(wc):
            i_out = 128 * c + m
            r = i_out // S + i_out % S  # padded row in [0, 68)
            rp = r - S2
            if 0 <= rp < H:
                sel[rp, c * 128 + m] = 1.0
    return sel.astype(ml_dtypes.bfloat16)


def kernel(x: np.ndarray, rand_vals: np.ndarray, **run_kwargs) -> np.ndarray:
    b, c, h, w = x.shape
    assert (b, c, h, w) == (4, 256, H, W)
    n_total = b * c

    keep = np.asarray(rand_vals) > RATE
    keep[S2::S, S2::S] = True
    m01 = keep.astype(np.float32).astype(ml_dtypes.bfloat16)
    sel = _build_sel()

    # fold dropout scale into x (single bf16 rounding), layout [h, n, w]
    xs = (np.asarray(x).reshape(n_total, h, w) * np.float32(SCALE)).astype(
        ml_dtypes.bfloat16
    )
    in_maps = []
    for k in range(N_CORES):
        xh = np.ascontiguousarray(
            xs[k * N : (k + 1) * N].transpose(1, 0, 2)
        ).reshape(H, N * W)
        in_maps.append({"xh": xh, "sel": sel, "mask": m01})

    nc = _get_nc()
    res = run_bass_kernel_spmd(nc, in_maps, core_ids=list(range(N_CORES)), **run_kwargs)
    _CACHE["last_results"] = res
    out = np.empty((n_total, OUT_HW, OUT_HW), dtype=np.float32)
    for k, r in enumerate(res.results):
        # device layout [I, n, J] -> [n, I, J], upcast bf16 -> f32
        out[k * N : (k + 1) * N] = r["out"].transpose(1, 0, 2)
    return out.reshape(b, c, OUT_HW, OUT_HW)
